# revision 3
# baseline (speedup 1.0000x reference)
"""EMA recurrent scan kernel for Trainium2 (Bass/Tile).

Computes h_t = |a|*x_t + (1-|a|)*h_{t-1} scanned over the T axis of a
[B=8, D=1024, T=4096] fp32 tensor, h_0 seeded from `hidden` [B, D, 1].

Sharding: batch dim (B=8) across the 8 NeuronCores — one [1024, 4096]
slab per core, no cross-core communication (recurrence is independent
per (b, d)).

Per-core kernel: for each of the 8 [128, 4096] partition tiles,
  1. DMA in (2 MiB contiguous rows, HWDGE via the SP ring)
  2. ACT: ax = a * x
  3. DVE tensor_tensor_scan: state = (1-a)*state + ax[:, t]  (fp32 state)
  4. DMA out via SWDGE (gpsimd) so store waits (gated on the late scan
     event) never block load issue on the SP ring — ~10 us faster (HW A/B)
Tile framework pipelines the stages across tiles (bufs=3).
"""

import numpy as np

import concourse.bass as bass
import concourse.mybir as mybir
from concourse import bass_utils, tile

ALPHA = 0.4
B, D, T = 8, 1024, 4096
N_CORES = 8
P = 128  # SBUF partitions
N_TILES = D // P  # 8 d-tiles per core


def _split_excess_waits(nc: bass.Bass) -> None:
    """The walrus build here allows only ONE sync-wait slot per instruction
    (and NONE on raw InstISA ops — codegen's visitInstISA rejects the added
    sync-wait command with "ISA wrong length").

    Tile's scheduler can attach several sem waits to one instruction; hoist
    the excess onto same-engine NoOps placed immediately before it
    (identical blocking semantics: the sequencer waits on each in order).
    """
    for f in nc.m.functions:
        for blk in f.blocks:
            new_insts = []
            changed = False
            for inst in blk.instructions:
                si = inst.sync_info
                keep = 0 if isinstance(inst, mybir.InstISA) else 1
                if si is not None and si.on_wait and len(si.on_wait) > keep:
                    waits = list(si.on_wait)
                    hoist = waits if keep == 0 else waits[:-1]
                    for k, w in enumerate(hoist):
                        new_insts.append(
                            mybir.InstNoOp(
                                name=f"{inst.name}.w{k}",
                                engine=inst.engine,
                                sync_info=mybir.SyncInfo(
                                    on_wait=[w], on_update=[]
                                ),
                                bass_nofuse=True,
                            )
                        )
                    inst.sync_info = mybir.SyncInfo(
                        on_wait=waits[len(hoist) :],
                        on_update=list(si.on_update),
                    )
                    changed = True
                new_insts.append(inst)
            if changed:
                blk.instructions = new_insts


def _build_nc(
    split_waits: bool = True,
    reps: int = 1,
    bufs: tuple[int, int, int] = (3, 3, 3),  # (x, ax, s)
    pool_store: bool = True,  # stores via SWDGE: ~7-12 us faster (HW A/B)
    unroll: bool = False,
) -> bass.Bass:
    a = abs(ALPHA)
    bx, bax, bs = bufs
    nc = bass.Bass(trn_type="TRN2")
    x = nc.dram_tensor("inp", [D, T], mybir.dt.float32, kind="ExternalInput")
    h = nc.dram_tensor("hidden", [D, 1], mybir.dt.float32, kind="ExternalInput")
    y = nc.dram_tensor("out", [D, T], mybir.dt.float32, kind="ExternalOutput")

    with tile.TileContext(nc) as tc:
        with (
            tc.tile_pool(name="const", bufs=1) as cpool,
            tc.tile_pool(name="io", bufs=3) as pool,
        ):
            # Constant (1-a) tile: data0 of the scan must match the free size.
            decay = cpool.tile([P, T], mybir.dt.float32)
            nc.vector.memset(decay[:, :], 1.0 - a)

            # All initial states in one small DMA: h0_all[p, i] = hidden[i*128+p, 0]
            h0_all = cpool.tile([P, N_TILES], mybir.dt.float32)
            nc.sync.dma_start(
                h0_all[:, :], h.rearrange("(t p) o -> p (t o)", p=P)
            )

            # Pre-scale structure: ACT computes ax = a*x right after each
            # load (runs ahead of the DVE), then the DVE scan produces the
            # final h_t = ax_t + (1-a)*h_{t-1} which is stored directly.
            # Keeps the serial DVE scan chain free of cross-engine hops:
            # its s-slot is released by the store DMA, ax arrives early.
            def body():
                for i in range(N_TILES):
                    xt = pool.tile([P, T], mybir.dt.float32, tag="x", name="xt", bufs=bx)
                    nc.sync.dma_start(xt[:, :], x[i * P : (i + 1) * P, :])

                    ax = pool.tile([P, T], mybir.dt.float32, tag="ax", name="ax", bufs=bax)
                    nc.scalar.mul(ax[:, :], xt[:, :], a)

                    s = pool.tile([P, T], mybir.dt.float32, tag="s", name="s", bufs=bs)
                    nc.vector.tensor_tensor_scan(
                        s[:, :],
                        decay[:, :],
                        ax[:, :],
                        h0_all[:, i : i + 1],
                        op0=mybir.AluOpType.mult,
                        op1=mybir.AluOpType.add,
                    )

                    # stores optionally via SWDGE (gpsimd) so their waits
                    # never block load issue on the SP HWDGE ring
                    store_eng = nc.gpsimd if pool_store else nc.sync
                    store_eng.dma_start(y[i * P : (i + 1) * P, :], s[:, :])

            if reps > 1 and not unroll:
                # bench-only: repeat the whole body in a dynamic loop so one
                # NEFF holds `reps` kernel executions (dispatch amortization)
                with tc.For_i(0, reps, 1):
                    body()
            elif reps > 1:
                for _ in range(reps):  # bench-only: straight-line repetition
                    body()
            else:
                body()

    if split_waits:
        _split_excess_waits(nc)
    return nc


_NC_CACHE: bass.Bass | None = None


def _get_nc() -> bass.Bass:
    global _NC_CACHE
    if _NC_CACHE is None:
        _NC_CACHE = _build_nc()
    return _NC_CACHE


def _make_in_maps(inp: np.ndarray, hidden: np.ndarray):
    inp = np.ascontiguousarray(np.asarray(inp, dtype=np.float32))
    hidden = np.ascontiguousarray(np.asarray(hidden, dtype=np.float32))
    assert inp.shape == (B, D, T), inp.shape
    assert hidden.shape == (B, D, 1), hidden.shape
    return [{"inp": inp[b], "hidden": hidden[b]} for b in range(N_CORES)]


def _run(inp: np.ndarray, hidden: np.ndarray, **spmd_kwargs):
    in_maps = _make_in_maps(inp, hidden)
    res = bass_utils.run_bass_kernel_spmd(
        _get_nc(), in_maps, core_ids=list(range(N_CORES)), **spmd_kwargs
    )
    out = np.stack([res.results[b]["out"] for b in range(N_CORES)], axis=0)
    return out, res


def kernel(inp: np.ndarray, hidden: np.ndarray) -> np.ndarray:
    out, _ = _run(inp, hidden)
    return out

